# revision 1
# baseline (speedup 1.0000x reference)
"""Trainium2 Bass kernel for nn_H_ATT (GatedTrans pair-attention block).

Math (per example):
  HE = tanh(hist@W_hy+b_hy) * lrelu(hist@W_hg+b_hg)      [R, H]
  QE = tanh(ques@W_qy+b_qy) * lrelu(ques@W_qg+b_qg)      [R, H]
  num[q,h]  = sum_k QE[q,k]*W_att[k]*HE[h,k]
  den[q,h]  = sqrt(sum_k QE[q,k]^2 * HE[h,k]^2)
  s = num / max(den, eps)          (b_att cancels in softmax)
  att = causal_softmax(s)          (softmax*tril/renorm == masked softmax)
  feat = att @ hist                 [R, 2H]

Sharding: pure data parallel, 8 examples per core on 8 NeuronCores.
The host pre-transposes activations and pre-blocks weights so every DMA is
contiguous-line friendly; the big GEMMs run as lhsT.T@rhs with K=IN on the
partition dim.
"""

import numpy as np
import ml_dtypes

import bass_rust
import concourse.bass as bass
import concourse.mybir as mybir
import concourse.tile as tile
from concourse.vector_clock import ScopedClock

# ---------------------------------------------------------------------------
# Workaround: this walrus build accepts only ONE semaphore wait on an SP
# Drain, but TileContext's tail drain carries one wait per live semaphore.
# Split them across a chain of drains.
# ---------------------------------------------------------------------------


def _patched_drain_and_barrier(self, tick_clock, wait_clock):
    nc = self.nc
    drain_inst = nc.sync.drain()
    wait_clock.add_sem_waits(
        drain_inst.ins, ScopedClock({None: tick_clock.global_clock})
    )
    waits = list(drain_inst.ins.sync_info.on_wait)
    if len(waits) > 1:
        drain_inst.ins.sync_info = bass_rust.SyncInfo(
            on_wait=waits[:1], on_update=list(drain_inst.ins.sync_info.on_update)
        )
        for i in range(1, len(waits)):
            extra = nc.sync.drain()
            extra.ins.sync_info = bass_rust.SyncInfo(
                on_wait=waits[i : i + 1], on_update=[]
            )
    nc.all_engine_barrier()
    assert self.sems is not None
    popped = nc._tile_sem_poison_stack.pop()
    assert popped is self._sem_poison
    nc.clear_and_free_semaphores(list(self.sems.allocated().values()))
    nc.all_engine_barrier()


tile.TileContext._drain_and_barrier = _patched_drain_and_barrier


def _split_multi_waits(nc):
    """This walrus build accepts at most one semaphore wait per instruction.
    Hoist extra waits onto standalone EventSemaphore instructions inserted
    just before the owning instruction in the same engine's stream."""
    uid = [0]
    for f in nc.m.functions:
        for bb in f.blocks:
            out = []
            for inst in bb.instructions:
                si = inst.sync_info
                if si is not None and len(si.on_wait) > 1:
                    waits = list(si.on_wait)
                    for w in waits[:-1]:
                        nop = mybir.InstEventSemaphore(
                            name=f"I-waitsplit-{uid[0]}", ins=[], outs=[]
                        )
                        uid[0] += 1
                        nop.engine = inst.engine
                        nop.sync_info = bass_rust.SyncInfo(
                            on_wait=[w], on_update=[]
                        )
                        out.append(nop)
                    inst.sync_info = bass_rust.SyncInfo(
                        on_wait=[waits[-1]], on_update=list(si.on_update)
                    )
                out.append(inst)
            bb.instructions[:] = out

# ---------------------------------------------------------------------------

B, R, H, IN = 64, 32, 1024, 2048
NCORES = 8
BL = B // NCORES  # examples per core
BR = BL * R  # 256 rows per core
KC = IN // 128  # 16 contraction chunks
MC = H // 128  # 8 h chunks
NEG = -1.0e30

F32 = mybir.dt.float32


def build_program(mode="f32r", zero_bias=True):
    """Build the per-core Bass program. mode in {"f32r", "bf16"} selects the
    dtype of the big-GEMM operands (weights + transposed activations)."""
    xdt = mybir.dt.float32r if mode == "f32r" else mybir.dt.bfloat16
    FEAT_DT = mybir.dt.float32r

    nc = bass.Bass()
    qt_d = nc.dram_tensor("qt", [KC, 128, BR], xdt, kind="ExternalInput")
    ht_d = nc.dram_tensor("ht", [KC, 128, BR], xdt, kind="ExternalInput")
    hn_d = nc.dram_tensor("hn", [2, 128, IN], FEAT_DT, kind="ExternalInput")
    wh_d = nc.dram_tensor("wh", [MC, 2, 128, KC, 128], xdt, kind="ExternalInput")
    wq_d = nc.dram_tensor("wq", [MC, 2, 128, KC, 128], xdt, kind="ExternalInput")
    b_d = {
        n: nc.dram_tensor(n, [128, MC], F32, kind="ExternalInput")
        for n in ("bhy", "bhg", "bqy", "bqg")
    }
    watt_d = nc.dram_tensor("watt", [128, MC], F32, kind="ExternalInput")
    mask_d = nc.dram_tensor("mask", [128, 128], F32, kind="ExternalInput")
    ident_d = nc.dram_tensor("ident", [128, 128], F32, kind="ExternalInput")
    feat_d = nc.dram_tensor("feat", [2, 128, IN], F32, kind="ExternalOutput")

    ACT = mybir.ActivationFunctionType

    with tile.TileContext(nc) as tc:
        with (
            tc.tile_pool(name="big", bufs=1) as big,
            tc.tile_pool(name="wts", bufs=5) as wts,
            tc.tile_pool(name="tmp", bufs=3) as tmp,
            tc.tile_pool(name="sm", bufs=1) as sm,
        ):
            # ques-transposed activations: needed first
            qt = big.tile([128, KC, BR], xdt, tag="qt")
            for q4 in range(4):
                ks = slice(4 * q4, 4 * (q4 + 1))
                nc.sync.dma_start(
                    qt[:, ks, :], qt_d[ks].rearrange("k p b -> p k b")
                )

            EDT = mybir.dt.bfloat16
            he = big.tile([128, MC, BR], EDT, tag="he")
            he2 = big.tile([128, MC, BR], EDT, tag="he2")
            qew = big.tile([128, MC, BR], EDT, tag="qew")
            qe2 = big.tile([128, MC, BR], EDT, tag="qe2")

            with (
                tc.tile_pool(name="pse", bufs=2, space="PSUM") as pse,
                tc.tile_pool(name="psnd", bufs=1, space="PSUM") as psnd,
            ):
                num_ps = [psnd.tile([128, 128], F32, name=f"num{g}", tag=f"num{g}") for g in range(2)]
                den_ps = [psnd.tile([128, 128], F32, name=f"den{g}", tag=f"den{g}") for g in range(2)]

                def gated(xt, w_dram, by, bg, m):
                    """One fused y+g weight DMA; returns (ty, tg) [128, BR]."""
                    wt = wts.tile([128, 2, KC, 128], xdt, tag="wt")
                    for h2 in range(2):
                        ks = slice(8 * h2, 8 * (h2 + 1))
                        nc.sync.dma_start(
                            wt[:, :, ks, :],
                            w_dram[m, :, :, ks].rearrange("y p k h -> p y k h"),
                        )
                    psy = pse.tile([128, BR], F32, tag="psy")
                    for k in range(KC):
                        nc.tensor.matmul(
                            psy[:], wt[:, 0, k, :], xt[:, k, :],
                            start=(k == 0), stop=(k == KC - 1),
                        )
                    psg = pse.tile([128, BR], F32, tag="psg")
                    for k in range(KC):
                        nc.tensor.matmul(
                            psg[:], wt[:, 1, k, :], xt[:, k, :],
                            start=(k == 0), stop=(k == KC - 1),
                        )
                    ty = tmp.tile([128, BR], F32, tag="ty")
                    nc.scalar.activation(ty[:], psy[:], ACT.Tanh, bias=by[:, m : m + 1])
                    t1 = tmp.tile([128, BR], F32, tag="t1")
                    tg = tmp.tile([128, BR], F32, tag="tg")
                    if zero_bias:
                        # leaky_relu(x) = max(x, 0.01x)
                        nc.vector.tensor_scalar_mul(t1[:], psg[:], 0.01)
                        nc.vector.tensor_max(tg[:], psg[:], t1[:])
                    else:
                        # leaky_relu(x+b) = max(x+b, 0.01*(x+b))
                        nc.vector.tensor_scalar(
                            t1[:], psg[:], bg[:, m : m + 1], 0.01,
                            op0=mybir.AluOpType.add, op1=mybir.AluOpType.mult,
                        )
                        nc.vector.tensor_scalar_add(tg[:], psg[:], bg[:, m : m + 1])
                        nc.vector.tensor_max(tg[:], tg[:], t1[:])
                    return ty, tg

                # consts land while the first ques matmuls run
                bsb = {}
                for n in ("bqy", "bqg", "bhy", "bhg"):
                    bsb[n] = sm.tile([128, MC], F32, name=n, tag=n)
                    nc.sync.dma_start(bsb[n][:], b_d[n][:])
                watt = sm.tile([128, MC], F32, tag="watt")
                nc.sync.dma_start(watt[:], watt_d[:])

                # ques embeddings (first: only needs qt + wq)
                for m in range(MC):
                    ty, tg = gated(qt, wq_d, bsb["bqy"], bsb["bqg"], m)
                    nc.vector.scalar_tensor_tensor(
                        qew[:, m, :], ty[:], watt[:, m : m + 1], tg[:],
                        op0=mybir.AluOpType.mult, op1=mybir.AluOpType.mult,
                    )
                    qe = tmp.tile([128, BR], F32, tag="qe")
                    nc.vector.tensor_mul(qe[:], ty[:], tg[:])
                    nc.scalar.square(qe2[:, m, :], qe[:])
                    if m == 0:
                        # hist-transposed activations: stream in during ques phase
                        ht = big.tile([128, KC, BR], xdt, tag="ht")
                        nc.sync.dma_start(ht[:], ht_d[:].rearrange("k p b -> p k b"))

                # hist embeddings + num/den accumulation per chunk
                for m in range(MC):
                    ty, tg = gated(ht, wh_d, bsb["bhy"], bsb["bhg"], m)
                    nc.vector.tensor_mul(he[:, m, :], ty[:], tg[:])
                    nc.scalar.square(he2[:, m, :], he[:, m, :])
                    for g in range(2):
                        sl = slice(128 * g, 128 * (g + 1))
                        nc.tensor.matmul(
                            num_ps[g][:], qew[:, m, sl], he[:, m, sl],
                            start=(m == 0), stop=(m == MC - 1),
                        )
                        nc.tensor.matmul(
                            den_ps[g][:], qe2[:, m, sl], he2[:, m, sl],
                            start=(m == 0), stop=(m == MC - 1),
                        )
                    if m == 0:
                        # feat inputs: stream in during hist phase
                        hn = big.tile([128, 2, IN], FEAT_DT, tag="hn")
                        nc.sync.dma_start(hn[:], hn_d[:].rearrange("t p d -> p t d"))
                        mask = sm.tile([128, 128], F32, tag="mask")
                        nc.sync.dma_start(mask[:], mask_d[:])
                        ident = sm.tile([128, 128], F32, tag="ident")
                        nc.sync.dma_start(ident[:], ident_d[:])

                # scores while num/den PSUM is still available
                sc = []
                for g in range(2):
                    sd = tmp.tile([128, 128], F32, tag="sd")
                    nc.scalar.activation(sd[:], den_ps[g][:], ACT.Sqrt)
                    rd = tmp.tile([128, 128], F32, tag="rd")
                    nc.vector.reciprocal(rd[:], sd[:])
                    s = sm.tile([128, 128], F32, name=f"sc{g}", tag=f"sc{g}")
                    nc.vector.tensor_mul(s[:], num_ps[g][:], rd[:])
                    nc.vector.tensor_add(s[:], s[:], mask[:])
                    sc.append(s)

            # attention tail + feat
            with (
                tc.tile_pool(name="psa", bufs=1, space="PSUM") as psa,
                tc.tile_pool(name="psf", bufs=4, space="PSUM") as psf,
            ):
                for g in range(2):
                    s = sc[g]
                    att = sm.tile([128, 128], F32, name=f"att{g}", tag=f"att{g}")
                    nc.vector.memset(att[:], 0.0)
                    rs = sm.tile([128, 1], F32, name=f"rs{g}", tag=f"rs{g}")
                    for e in range(4):
                        bl = slice(32 * e, 32 * (e + 1))
                        nc.scalar.activation(att[bl, bl], s[bl, bl], ACT.Exp)
                        nc.vector.reduce_sum(
                            rs[bl, :], att[bl, bl], axis=mybir.AxisListType.X
                        )
                    rrs = sm.tile([128, 1], F32, name=f"rrs{g}", tag=f"rrs{g}")
                    nc.vector.reciprocal(rrs[:], rs[:])
                    nc.vector.tensor_scalar_mul(att[:], att[:], rrs[:])
                    atp = psa.tile([128, 128], F32, tag="atp")
                    nc.tensor.transpose(atp[:], att[:], ident[:])
                    atb = sm.tile([128, 128], FEAT_DT, name=f"atb{g}", tag=f"atb{g}")
                    nc.scalar.copy(atb[:], atp[:])
                    for c2 in range(2):
                        fsb = tmp.tile([128, 1024], F32, tag="fsb")
                        for half in range(2):
                            c = 2 * c2 + half
                            cs = slice(512 * c, 512 * (c + 1))
                            fps = psf.tile([128, 512], F32, tag="fps")
                            nc.tensor.matmul(
                                fps[:], atb[:], hn[:, g, cs], start=True, stop=True
                            )
                            dst = fsb[:, 512 * half : 512 * (half + 1)]
                            if half == 0:
                                nc.scalar.copy(dst, fps[:])
                            else:
                                nc.vector.tensor_copy(dst, fps[:])
                        nc.sync.dma_start(
                            feat_d[g, :, 1024 * c2 : 1024 * (c2 + 1)], fsb[:]
                        )

    _split_multi_waits(nc)
    return nc


# ---------------------------------------------------------------------------
# Host side
# ---------------------------------------------------------------------------

_PROG_CACHE = {}


def _get_prog(mode, zero_bias):
    key = (mode, zero_bias)
    if key not in _PROG_CACHE:
        _PROG_CACHE[key] = build_program(mode, zero_bias)
    return _PROG_CACHE[key]


def _prep_shared(W_hy, b_hy, W_hg, b_hg, W_qy, b_qy, W_qg, b_qg, W_att, mode):
    xnp = np.float32 if mode == "f32r" else ml_dtypes.bfloat16

    def reblock(W):
        # [IN, H] -> [MC, 128, KC, 128]; Wr[m, p, k, h] = W[128k+p, 128m+h]
        return np.ascontiguousarray(
            W.reshape(KC, 128, MC, 128).transpose(2, 1, 0, 3)
        ).astype(xnp)

    def bvec(b):
        return np.ascontiguousarray(b.reshape(MC, 128).T).astype(np.float32)

    m32 = np.where(
        np.arange(32)[None, :] <= np.arange(32)[:, None], 0.0, NEG
    ).astype(np.float32)
    mask = np.tile(m32, (4, 4))
    wh = np.ascontiguousarray(np.stack([reblock(W_hy), reblock(W_hg)], axis=1))
    wq = np.ascontiguousarray(np.stack([reblock(W_qy), reblock(W_qg)], axis=1))
    shared = {
        "wh": wh,
        "wq": wq,
        "bhy": bvec(b_hy),
        "bhg": bvec(b_hg),
        "bqy": bvec(b_qy),
        "bqg": bvec(b_qg),
        "watt": bvec(W_att),
        "mask": np.ascontiguousarray(mask),
        "ident": np.eye(128, dtype=np.float32),
    }
    return shared, xnp


def kernel(
    hist, ques, W_hy, b_hy, W_hg, b_hg, W_qy, b_qy, W_qg, b_qg, W_att, b_att,
    mode="f32r", trace=False,
):
    from concourse.bass_utils import run_bass_kernel_spmd

    hist = np.asarray(hist, np.float32)
    ques = np.asarray(ques, np.float32)
    zero_bias = all(
        not np.any(np.asarray(b)) for b in (b_hy, b_hg, b_qy, b_qg)
    )
    nc = _get_prog(mode, zero_bias)
    shared, xnp = _prep_shared(
        np.asarray(W_hy, np.float32), np.asarray(b_hy, np.float32),
        np.asarray(W_hg, np.float32), np.asarray(b_hg, np.float32),
        np.asarray(W_qy, np.float32), np.asarray(b_qy, np.float32),
        np.asarray(W_qg, np.float32), np.asarray(b_qg, np.float32),
        np.asarray(W_att, np.float32), mode,
    )
    in_maps = []
    for c in range(NCORES):
        hs = hist[c * BL : (c + 1) * BL].reshape(BR, IN)
        qs = ques[c * BL : (c + 1) * BL].reshape(BR, IN)
        im = dict(shared)
        im["qt"] = np.ascontiguousarray(qs.T).reshape(KC, 128, BR).astype(xnp)
        im["ht"] = np.ascontiguousarray(hs.T).reshape(KC, 128, BR).astype(xnp)
        im["hn"] = np.ascontiguousarray(hs.reshape(2, 128, IN))
        in_maps.append(im)

    res = run_bass_kernel_spmd(
        nc, in_maps, core_ids=list(range(NCORES)), trace=trace
    )
    feat = np.concatenate(
        [r["feat"].reshape(BL, R, IN) for r in res.results], axis=0
    )
    if trace:
        return feat, res
    return feat



# revision 5
# speedup vs baseline: 1.5182x; 1.5182x over previous
"""Trainium2 Bass kernel for nn_H_ATT (GatedTrans pair-attention block).

Math (per example):
  HE = tanh(hist@W_hy+b_hy) * lrelu(hist@W_hg+b_hg)      [R, H]
  QE = tanh(ques@W_qy+b_qy) * lrelu(ques@W_qg+b_qg)      [R, H]
  numT[h,q] = sum_k HE[h,k]*QE[q,k]*W_att[k]
  denT[h,q] = sqrt(sum_k HE[h,k]^2 * QE[q,k]^2)
  sT = numT / den                   (b_att cancels in softmax)
  attU^T = exp(sT + maskT)          (causal mask additive; unnormalized)
  feat = (attU @ hist) / rowsum(attU)    [R, 2H]

Sharding: pure data parallel, 8 examples per core on 8 NeuronCores.
Default mode quantizes the four big [IN,H] weight matrices and the
activations to fp8 e4m3 (weights pre-scaled by 1024 to clear the e4m3
subnormal range; the 1/1024 descale is folded into the tanh/relu
activations) and runs the embedding GEMMs in DoubleRow perf mode.
The score/attention path stays in bf16 with fp32 PSUM accumulation.
All host-side layouts are partition-major so every DMA is a straight
[128, contiguous-bytes] copy.
"""

import numpy as np
import ml_dtypes

import bass_rust
import concourse.bass as bass
import concourse.mybir as mybir
import concourse.tile as tile
from concourse.vector_clock import ScopedClock

# ---------------------------------------------------------------------------
# Workaround: this walrus build accepts only ONE semaphore wait on an SP
# Drain, but TileContext's tail drain carries one wait per live semaphore.
# Split them across a chain of drains.
# ---------------------------------------------------------------------------


def _patched_drain_and_barrier(self, tick_clock, wait_clock):
    nc = self.nc
    drain_inst = nc.sync.drain()
    wait_clock.add_sem_waits(
        drain_inst.ins, ScopedClock({None: tick_clock.global_clock})
    )
    waits = list(drain_inst.ins.sync_info.on_wait)
    if len(waits) > 1:
        drain_inst.ins.sync_info = bass_rust.SyncInfo(
            on_wait=waits[:1], on_update=list(drain_inst.ins.sync_info.on_update)
        )
        for i in range(1, len(waits)):
            extra = nc.sync.drain()
            extra.ins.sync_info = bass_rust.SyncInfo(
                on_wait=waits[i : i + 1], on_update=[]
            )
    nc.all_engine_barrier()
    assert self.sems is not None
    popped = nc._tile_sem_poison_stack.pop()
    assert popped is self._sem_poison
    nc.clear_and_free_semaphores(list(self.sems.allocated().values()))
    nc.all_engine_barrier()


tile.TileContext._drain_and_barrier = _patched_drain_and_barrier


def _split_multi_waits(nc):
    """This walrus build accepts at most one semaphore wait per instruction.
    Hoist extra waits onto standalone EventSemaphore instructions inserted
    just before the owning instruction in the same engine's stream."""
    uid = [0]
    for f in nc.m.functions:
        for bb in f.blocks:
            out = []
            for inst in bb.instructions:
                si = inst.sync_info
                if si is not None and len(si.on_wait) > 1:
                    waits = list(si.on_wait)
                    for w in waits[:-1]:
                        nop = mybir.InstEventSemaphore(
                            name=f"I-waitsplit-{uid[0]}", ins=[], outs=[]
                        )
                        uid[0] += 1
                        nop.engine = inst.engine
                        nop.sync_info = bass_rust.SyncInfo(
                            on_wait=[w], on_update=[]
                        )
                        out.append(nop)
                    inst.sync_info = bass_rust.SyncInfo(
                        on_wait=[waits[-1]], on_update=list(si.on_update)
                    )
                out.append(inst)
            bb.instructions[:] = out

# ---------------------------------------------------------------------------

B, R, H, IN = 64, 32, 1024, 2048
NCORES = 8
BL = B // NCORES  # examples per core
BR = BL * R  # 256 rows per core
KC = IN // 128  # 16 contraction chunks
MC = H // 128  # 8 h chunks
NEG = -1.0e30
WSCALE = 1024.0  # fp8 weight pre-scale

F32 = mybir.dt.float32
BF16 = mybir.dt.bfloat16
FP8 = mybir.dt.float8e4


def build_program(mode="fp8", zero_bias=True):
    fp8 = mode == "fp8"
    xdt = FP8 if fp8 else BF16
    s = (1.0 / WSCALE) if fp8 else 1.0
    EDT = BF16

    nc = bass.Bass()
    qt_d = nc.dram_tensor("qt", [128, KC, BR], xdt, kind="ExternalInput")
    ht_d = nc.dram_tensor("ht", [128, KC, BR], xdt, kind="ExternalInput")
    hn_d = nc.dram_tensor("hn", [128, 2, IN], BF16, kind="ExternalInput")
    wh_d = nc.dram_tensor("wh", [MC, 128, 2, KC, 128], xdt, kind="ExternalInput")
    wq_d = nc.dram_tensor("wq", [MC, 128, 2, KC, 128], xdt, kind="ExternalInput")
    b_d = {
        n: nc.dram_tensor(n, [128, MC], F32, kind="ExternalInput")
        for n in ("bhy", "bhg", "bqy", "bqg")
    }
    watt_d = nc.dram_tensor("watt", [128, MC], F32, kind="ExternalInput")
    maskT_d = nc.dram_tensor("maskT", [128, 128], F32, kind="ExternalInput")
    feat_d = nc.dram_tensor("feat", [2, 128, IN], BF16, kind="ExternalOutput")

    ACT = mybir.ActivationFunctionType
    ALU = mybir.AluOpType

    with tile.TileContext(nc) as tc:
        with (
            tc.tile_pool(name="big", bufs=1) as big,
            tc.tile_pool(name="wts", bufs=4) as wts,
            tc.tile_pool(name="tmp", bufs=3) as tmp,
            tc.tile_pool(name="sm", bufs=1) as sm,
        ):
            # ques-transposed activations: needed first (2 DMAs, k halves)
            qt = big.tile([128, KC, BR], xdt, tag="qt")
            nc.sync.dma_start(qt[:, : KC // 2, :], qt_d[:, : KC // 2, :])
            nc.sync.dma_start(qt[:, KC // 2 :, :], qt_d[:, KC // 2 :, :])

            he = big.tile([128, MC, BR], EDT, tag="he")
            he2 = big.tile([128, MC, BR], EDT, tag="he2")
            qew = big.tile([128, MC, BR], EDT, tag="qew")
            qe2 = big.tile([128, MC, BR], EDT, tag="qe2")

            with (
                tc.tile_pool(name="pse", bufs=2, space="PSUM") as pse,
                tc.tile_pool(name="psnd", bufs=1, space="PSUM") as psnd,
            ):
                numT_ps = [
                    psnd.tile([128, 128], F32, name=f"num{g}", tag=f"num{g}")
                    for g in range(2)
                ]
                denT_ps = [
                    psnd.tile([128, 128], F32, name=f"den{g}", tag=f"den{g}")
                    for g in range(2)
                ]

                def embed_mm(ps, wt, xt):
                    if fp8:
                        for j in range(KC // 2):
                            nc.tensor.matmul(
                                ps[:],
                                wt[:, 2 * j : 2 * j + 2, :],
                                xt[:, 2 * j : 2 * j + 2, :],
                                start=(j == 0),
                                stop=(j == KC // 2 - 1),
                                perf_mode=mybir.MatmulPerfMode.DoubleRow,
                            )
                    else:
                        for k in range(KC):
                            nc.tensor.matmul(
                                ps[:], wt[:, k, :], xt[:, k, :],
                                start=(k == 0), stop=(k == KC - 1),
                            )

                def gated(xt, w_dram, by, bg, m):
                    """Per-unit weight DMAs; returns (ty, tg) [128, BR] bf16."""
                    wty = wts.tile([128, KC, 128], xdt, tag="wty")
                    nc.sync.dma_start(wty[:], w_dram[m, :, 0])
                    wtg = wts.tile([128, KC, 128], xdt, tag="wtg")
                    nc.sync.dma_start(wtg[:], w_dram[m, :, 1])
                    psy = pse.tile([128, BR], F32, tag="psy")
                    embed_mm(psy, wty, xt)
                    psg = pse.tile([128, BR], F32, tag="psg")
                    embed_mm(psg, wtg, xt)
                    ty = tmp.tile([128, BR], EDT, tag="ty")
                    nc.scalar.activation(
                        ty[:], psy[:], ACT.Tanh,
                        bias=(0.0 if zero_bias else by[:, m : m + 1]), scale=s,
                    )
                    tg = tmp.tile([128, BR], EDT, tag="tg")
                    if zero_bias:
                        # lrelu(s*z) = 0.01*s*z + 0.99*relu(s*z)
                        r = tmp.tile([128, BR], F32, tag="r")
                        nc.scalar.activation(r[:], psg[:], ACT.Relu, scale=0.99 * s)
                        nc.vector.scalar_tensor_tensor(
                            tg[:], psg[:], 0.01 * s, r[:],
                            op0=ALU.mult, op1=ALU.add,
                        )
                    else:
                        # lrelu(s*z + b) = max(a, 0.01a), a = s*z + b
                        a = tmp.tile([128, BR], F32, tag="r")
                        nc.scalar.activation(
                            a[:], psg[:], ACT.Identity,
                            bias=bg[:, m : m + 1], scale=s,
                        )
                        t1 = tmp.tile([128, BR], F32, tag="t1")
                        nc.gpsimd.tensor_scalar_mul(t1[:], a[:], 0.01)
                        nc.vector.tensor_max(tg[:], a[:], t1[:])
                    return ty, tg

                # consts land while the first ques matmuls run
                bsb = {}
                for n in ("bqy", "bqg", "bhy", "bhg"):
                    bsb[n] = sm.tile([128, MC], F32, name=n, tag=n)
                    nc.sync.dma_start(bsb[n][:], b_d[n][:])
                watt = sm.tile([128, MC], F32, tag="watt")
                nc.sync.dma_start(watt[:], watt_d[:])
                maskT = sm.tile([128, 128], F32, tag="maskT")
                nc.sync.dma_start(maskT[:], maskT_d[:])
                ones = sm.tile([128, 1], BF16, tag="ones")
                nc.vector.memset(ones[:], 1.0)

                # ques embeddings
                for m in range(MC):
                    ty, tg = gated(qt, wq_d, bsb["bqy"], bsb["bqg"], m)
                    nc.vector.scalar_tensor_tensor(
                        qew[:, m, :], ty[:], watt[:, m : m + 1], tg[:],
                        op0=ALU.mult, op1=ALU.mult,
                    )
                    qe = tmp.tile([128, BR], EDT, tag="qe")
                    nc.gpsimd.tensor_mul(qe[:], ty[:], tg[:])
                    nc.gpsimd.tensor_mul(qe2[:, m, :], qe[:], qe[:])
                    if m == 0:
                        # hist-transposed activations: stream during ques phase
                        ht = big.tile([128, KC, BR], xdt, tag="ht")
                        nc.sync.dma_start(ht[:, : KC // 2, :], ht_d[:, : KC // 2, :])
                        nc.sync.dma_start(ht[:, KC // 2 :, :], ht_d[:, KC // 2 :, :])
                    if m == 2:
                        # feat inputs: stream well before the tail needs them
                        hn = big.tile([128, 2, IN], BF16, tag="hn")
                        nc.sync.dma_start(hn[:], hn_d[:])

                # hist embeddings + transposed num/den accumulation.
                # numT/denT matmuls for chunk m are emitted during chunk m+1's
                # embedding matmuls so the tensor engine never waits on the
                # vector engine's he/he2 production.
                def numden(m):
                    for g in range(2):
                        sl = slice(128 * g, 128 * (g + 1))
                        nc.tensor.matmul(
                            numT_ps[g][:], he[:, m, sl], qew[:, m, sl],
                            start=(m == 0), stop=(m == MC - 1),
                        )
                        nc.tensor.matmul(
                            denT_ps[g][:], he2[:, m, sl], qe2[:, m, sl],
                            start=(m == 0), stop=(m == MC - 1),
                        )

                for m in range(MC):
                    ty, tg = gated(ht, wh_d, bsb["bhy"], bsb["bhg"], m)
                    nc.vector.tensor_mul(he[:, m, :], ty[:], tg[:])
                    nc.gpsimd.tensor_mul(he2[:, m, :], he[:, m, :], he[:, m, :])
                    if m > 0:
                        numden(m - 1)
                numden(MC - 1)

                # masked scores while num/den PSUM is still allocated
                sc = []
                for g in range(2):
                    sd = tmp.tile([128, 128], F32, tag="sd")
                    nc.scalar.activation(sd[:], denT_ps[g][:], ACT.Sqrt)
                    rdT = tmp.tile([128, 128], F32, tag="rdT")
                    nc.vector.reciprocal(rdT[:], sd[:])
                    sT = sm.tile([128, 128], F32, name=f"sT{g}", tag=f"sT{g}")
                    nc.vector.tensor_mul(sT[:], numT_ps[g][:], rdT[:])
                    nc.vector.tensor_add(sT[:], sT[:], maskT[:])
                    sc.append(sT)

            # attention tail: attU^T = exp(sT), row sums via a ones-column
            # matmul, 1/rowsum folded into the output copy.
            with (
                tc.tile_pool(name="psa", bufs=2, space="PSUM") as psa,
                tc.tile_pool(name="psf", bufs=2, space="PSUM") as psf,
            ):
                for g in range(2):
                    attT = sm.tile([128, 128], EDT, name=f"attT{g}", tag=f"attT{g}")
                    nc.scalar.activation(attT[:], sc[g][:], ACT.Exp)
                    rs_ps = psa.tile([128, 1], F32, tag="rs")
                    nc.tensor.matmul(rs_ps[:], attT[:], ones[:])
                    rrs = sm.tile([128, 1], F32, name=f"rrs{g}", tag=f"rrs{g}")
                    nc.vector.reciprocal(rrs[:], rs_ps[:])
                    for c2 in range(2):
                        fsb = tmp.tile([128, 1024], BF16, tag="fsb")
                        for half in range(2):
                            c = 2 * c2 + half
                            cs = slice(512 * c, 512 * (c + 1))
                            fps = psf.tile([128, 512], F32, tag="fps")
                            nc.tensor.matmul(
                                fps[:], attT[:], hn[:, g, cs],
                                start=True, stop=True,
                            )
                            dst = fsb[:, 512 * half : 512 * (half + 1)]
                            if half == 0:
                                nc.scalar.activation(
                                    dst, fps[:], ACT.Copy, scale=rrs[:]
                                )
                            else:
                                nc.vector.tensor_scalar_mul(dst, fps[:], rrs[:])
                        nc.sync.dma_start(
                            feat_d[g, :, 1024 * c2 : 1024 * (c2 + 1)], fsb[:]
                        )

    _split_multi_waits(nc)
    return nc


# ---------------------------------------------------------------------------
# Host side
# ---------------------------------------------------------------------------

_PROG_CACHE = {}


def _get_prog(mode, zero_bias):
    key = (mode, zero_bias)
    if key not in _PROG_CACHE:
        _PROG_CACHE[key] = build_program(mode, zero_bias)
    return _PROG_CACHE[key]


def _prep_shared(W_hy, b_hy, W_hg, b_hg, W_qy, b_qy, W_qg, b_qg, W_att, mode):
    fp8 = mode == "fp8"
    xnp = ml_dtypes.float8_e4m3 if fp8 else ml_dtypes.bfloat16
    ws = WSCALE if fp8 else 1.0

    def reblock(W):
        # [IN, H] -> [MC, 128, KC, 128]; Wr[m, p, k, h] = W[128k+p, 128m+h]
        return np.ascontiguousarray(
            (W * ws).reshape(KC, 128, MC, 128).transpose(2, 1, 0, 3)
        ).astype(xnp)

    def bvec(b):
        return np.ascontiguousarray(b.reshape(MC, 128).T).astype(np.float32)

    # transposed causal mask: maskT[h, q] = 0 if h <= q (same example), -inf-ish
    # otherwise; off-diagonal 32x32 blocks fully masked.
    maskT = np.full((128, 128), NEG, np.float32)
    m32T = np.where(
        np.arange(32)[:, None] <= np.arange(32)[None, :], 0.0, NEG
    ).astype(np.float32)
    for e in range(4):
        maskT[32 * e : 32 * (e + 1), 32 * e : 32 * (e + 1)] = m32T

    # [MC, 128, 2, KC, 128]
    wh = np.ascontiguousarray(np.stack([reblock(W_hy), reblock(W_hg)], axis=2))
    wq = np.ascontiguousarray(np.stack([reblock(W_qy), reblock(W_qg)], axis=2))
    shared = {
        "wh": wh,
        "wq": wq,
        "bhy": bvec(b_hy),
        "bhg": bvec(b_hg),
        "bqy": bvec(b_qy),
        "bqg": bvec(b_qg),
        "watt": bvec(W_att),
        "maskT": np.ascontiguousarray(maskT),
    }
    return shared, xnp


def kernel(
    hist, ques, W_hy, b_hy, W_hg, b_hg, W_qy, b_qy, W_qg, b_qg, W_att, b_att,
    mode="fp8", trace=False,
):
    from concourse.bass_utils import run_bass_kernel_spmd

    hist = np.asarray(hist, np.float32)
    ques = np.asarray(ques, np.float32)
    zero_bias = all(
        not np.any(np.asarray(b)) for b in (b_hy, b_hg, b_qy, b_qg)
    )
    nc = _get_prog(mode, zero_bias)
    shared, xnp = _prep_shared(
        np.asarray(W_hy, np.float32), np.asarray(b_hy, np.float32),
        np.asarray(W_hg, np.float32), np.asarray(b_hg, np.float32),
        np.asarray(W_qy, np.float32), np.asarray(b_qy, np.float32),
        np.asarray(W_qg, np.float32), np.asarray(b_qg, np.float32),
        np.asarray(W_att, np.float32), mode,
    )

    def pmaj(x2d):
        # [BR, IN] -> [128, KC, BR]: out[p, k, b] = x2d[b, 128k+p]
        return np.ascontiguousarray(
            x2d.T.reshape(KC, 128, BR).transpose(1, 0, 2)
        ).astype(xnp)

    in_maps = []
    for c in range(NCORES):
        hs = hist[c * BL : (c + 1) * BL].reshape(BR, IN)
        qs = ques[c * BL : (c + 1) * BL].reshape(BR, IN)
        im = dict(shared)
        im["qt"] = pmaj(qs)
        im["ht"] = pmaj(hs)
        im["hn"] = np.ascontiguousarray(
            hs.reshape(2, 128, IN).transpose(1, 0, 2)
        ).astype(ml_dtypes.bfloat16)
        in_maps.append(im)

    res = run_bass_kernel_spmd(
        nc, in_maps, core_ids=list(range(NCORES)), trace=trace
    )
    feat = np.concatenate(
        [
            r["feat"].astype(np.float32).reshape(BR, IN).reshape(BL, R, IN)
            for r in res.results
        ],
        axis=0,
    )
    if trace:
        return feat, res
    return feat


# revision 11
# speedup vs baseline: 1.5644x; 1.0304x over previous
"""Trainium2 Bass kernel for nn_H_ATT (GatedTrans pair-attention block).

Math (per example):
  HE = tanh(hist@W_hy+b_hy) * lrelu(hist@W_hg+b_hg)      [R, H]
  QE = tanh(ques@W_qy+b_qy) * lrelu(ques@W_qg+b_qg)      [R, H]
  numT[h,q] = sum_k HE[h,k]*QE[q,k]*W_att[k]
  denT[h,q] = sqrt(sum_k HE[h,k]^2 * QE[q,k]^2)
  sT = numT / den                   (b_att cancels in softmax)
  attU^T = exp(sT + maskT)          (causal mask additive; unnormalized)
  feat = (attU @ hist) / rowsum(attU)    [R, 2H]

Sharding: pure data parallel, 8 examples per core on 8 NeuronCores.
Default mode quantizes the four big [IN,H] weight matrices and the
activations to fp8 e4m3 (weights pre-scaled by 1024 to clear the e4m3
subnormal range; the 1/1024 descale is folded into the tanh/relu
activations) and runs the embedding GEMMs in DoubleRow perf mode.
The score/attention path stays in bf16 with fp32 PSUM accumulation.
All host-side layouts are partition-major so every DMA is a straight
[128, contiguous-bytes] copy.
"""

import numpy as np
import ml_dtypes

import bass_rust
import concourse.bass as bass
import concourse.mybir as mybir
import concourse.tile as tile
from concourse.vector_clock import ScopedClock

# ---------------------------------------------------------------------------
# Workaround: this walrus build accepts only ONE semaphore wait on an SP
# Drain, but TileContext's tail drain carries one wait per live semaphore.
# Split them across a chain of drains.
# ---------------------------------------------------------------------------


def _patched_drain_and_barrier(self, tick_clock, wait_clock):
    nc = self.nc
    drain_inst = nc.sync.drain()
    wait_clock.add_sem_waits(
        drain_inst.ins, ScopedClock({None: tick_clock.global_clock})
    )
    waits = list(drain_inst.ins.sync_info.on_wait)
    if len(waits) > 1:
        drain_inst.ins.sync_info = bass_rust.SyncInfo(
            on_wait=waits[:1], on_update=list(drain_inst.ins.sync_info.on_update)
        )
        for i in range(1, len(waits)):
            extra = nc.sync.drain()
            extra.ins.sync_info = bass_rust.SyncInfo(
                on_wait=waits[i : i + 1], on_update=[]
            )
    nc.all_engine_barrier()
    assert self.sems is not None
    popped = nc._tile_sem_poison_stack.pop()
    assert popped is self._sem_poison
    nc.clear_and_free_semaphores(list(self.sems.allocated().values()))
    nc.all_engine_barrier()


tile.TileContext._drain_and_barrier = _patched_drain_and_barrier


def _split_multi_waits(nc):
    """This walrus build accepts at most one semaphore wait per instruction.
    Hoist extra waits onto standalone EventSemaphore instructions inserted
    just before the owning instruction in the same engine's stream."""
    uid = [0]
    for f in nc.m.functions:
        for bb in f.blocks:
            out = []
            for inst in bb.instructions:
                si = inst.sync_info
                if si is not None and len(si.on_wait) > 1:
                    waits = list(si.on_wait)
                    for w in waits[:-1]:
                        nop = mybir.InstEventSemaphore(
                            name=f"I-waitsplit-{uid[0]}", ins=[], outs=[]
                        )
                        uid[0] += 1
                        nop.engine = inst.engine
                        nop.sync_info = bass_rust.SyncInfo(
                            on_wait=[w], on_update=[]
                        )
                        out.append(nop)
                    inst.sync_info = bass_rust.SyncInfo(
                        on_wait=[waits[-1]], on_update=list(si.on_update)
                    )
                out.append(inst)
            bb.instructions[:] = out

# ---------------------------------------------------------------------------

B, R, H, IN = 64, 32, 1024, 2048
NCORES = 8
BL = B // NCORES  # examples per core
BR = BL * R  # 256 rows per core
KC = IN // 128  # 16 contraction chunks
MC = H // 128  # 8 h chunks
NEG = -1.0e30
WSCALE = 1024.0  # fp8 weight pre-scale

F32 = mybir.dt.float32
BF16 = mybir.dt.bfloat16
FP8 = mybir.dt.float8e4


def build_program(mode="fp8", zero_bias=True):
    fp8 = mode == "fp8"
    xdt = FP8 if fp8 else BF16
    s = (1.0 / WSCALE) if fp8 else 1.0
    EDT = BF16

    nc = bass.Bass()
    qt_d = nc.dram_tensor("qt", [128, KC, BR], xdt, kind="ExternalInput")
    ht_d = nc.dram_tensor("ht", [128, KC, BR], xdt, kind="ExternalInput")
    hn_d = nc.dram_tensor("hn", [128, 2, IN], BF16, kind="ExternalInput")
    wh_d = nc.dram_tensor("wh", [MC, 128, 2, KC, 128], xdt, kind="ExternalInput")
    wq_d = nc.dram_tensor("wq", [MC, 128, 2, KC, 128], xdt, kind="ExternalInput")
    # packed consts: bqy|bqg|bhy|bhg|watt (5*MC cols) then maskT (128 cols)
    consts_d = nc.dram_tensor("consts", [128, 5 * MC + 128], F32, kind="ExternalInput")
    feat_d = nc.dram_tensor("feat", [2, 128, IN], BF16, kind="ExternalOutput")

    ACT = mybir.ActivationFunctionType
    ALU = mybir.AluOpType

    with tile.TileContext(nc) as tc:
        with (
            tc.tile_pool(name="big", bufs=1) as big,
            tc.tile_pool(name="wts", bufs=4) as wts,
            tc.tile_pool(name="tmp", bufs=3) as tmp,
            tc.tile_pool(name="sm", bufs=1) as sm,
        ):
            # ques-transposed activations + first weights: trigger order is
            # critical-path order (SP issues DMA triggers serially).
            qt = big.tile([128, KC, BR], xdt, tag="qt")
            nc.sync.dma_start(qt[:], qt_d[:])

            pending = {}

            def wload(which, w_dram, m):
                wt = wts.tile([128, 2, KC, 128], xdt, tag="wt")
                nc.sync.dma_start(wt[:], w_dram[m])
                pending[(which, m)] = wt

            wload("q", wq_d, 0)
            wload("q", wq_d, 1)

            consts = sm.tile([128, 5 * MC + 128], F32, tag="consts")
            nc.sync.dma_start(consts[:], consts_d[:])
            bsb = {
                n: consts[:, i * MC : (i + 1) * MC]
                for i, n in enumerate(("bqy", "bqg", "bhy", "bhg"))
            }
            watt = consts[:, 4 * MC : 5 * MC]
            maskT = consts[:, 5 * MC :]

            # pre-warm the scalar engine's activation tables (Tanh/Relu,
            # Sqrt, Exp) so no ACT_TABLE_LOAD lands on the critical path.
            warm = sm.tile([128, 1], F32, tag="warm")
            nc.vector.memset(warm[:], 0.5)
            wout = sm.tile([128, 1], F32, tag="wout")
            for fn in (ACT.Tanh, ACT.Relu, ACT.Sqrt, ACT.Exp):
                nc.scalar.activation(wout[:], warm[:], fn)
            ones = sm.tile([128, 1], BF16, tag="ones")
            nc.vector.memset(ones[:], 1.0)

            he = big.tile([128, MC, BR], EDT, tag="he")
            he2 = big.tile([128, MC, BR], EDT, tag="he2")
            qew = big.tile([128, MC, BR], EDT, tag="qew")
            qe2 = big.tile([128, MC, BR], EDT, tag="qe2")

            with (
                tc.tile_pool(name="pse", bufs=2, space="PSUM") as pse,
                tc.tile_pool(name="psnd", bufs=1, space="PSUM") as psnd,
            ):
                numT_ps = [
                    psnd.tile([128, 128], F32, name=f"num{g}", tag=f"num{g}")
                    for g in range(2)
                ]
                denT_ps = [
                    psnd.tile([128, 128], F32, name=f"den{g}", tag=f"den{g}")
                    for g in range(2)
                ]

                def embed_mm(ps, wt, xt):
                    if fp8:
                        for j in range(KC // 2):
                            nc.tensor.matmul(
                                ps[:],
                                wt[:, 2 * j : 2 * j + 2, :],
                                xt[:, 2 * j : 2 * j + 2, :],
                                start=(j == 0),
                                stop=(j == KC // 2 - 1),
                                perf_mode=mybir.MatmulPerfMode.DoubleRow,
                            )
                    else:
                        for k in range(KC):
                            nc.tensor.matmul(
                                ps[:], wt[:, k, :], xt[:, k, :],
                                start=(k == 0), stop=(k == KC - 1),
                            )

                def gated(xt, which, by, bg, m):
                    """Consumes the prefetched weight tile for (which, m)."""
                    wt = pending.pop((which, m))
                    psy = pse.tile([128, BR], F32, tag="psy")
                    embed_mm(psy, wt[:, 0], xt)
                    psg = pse.tile([128, BR], F32, tag="psg")
                    embed_mm(psg, wt[:, 1], xt)
                    ty = tmp.tile([128, BR], EDT, tag="ty")
                    nc.scalar.activation(
                        ty[:], psy[:], ACT.Tanh,
                        bias=(0.0 if zero_bias else by[:, m : m + 1]), scale=s,
                    )
                    tg = tmp.tile([128, BR], EDT, tag="tg")
                    if zero_bias:
                        # lrelu(s*z) = 0.01*s*z + 0.99*relu(s*z)
                        r = tmp.tile([128, BR], F32, tag="r")
                        nc.scalar.activation(r[:], psg[:], ACT.Relu, scale=0.99 * s)
                        nc.vector.scalar_tensor_tensor(
                            tg[:], psg[:], 0.01 * s, r[:],
                            op0=ALU.mult, op1=ALU.add,
                        )
                    else:
                        # lrelu(s*z + b) = max(a, 0.01a), a = s*z + b
                        a = tmp.tile([128, BR], F32, tag="r")
                        nc.scalar.activation(
                            a[:], psg[:], ACT.Identity,
                            bias=bg[:, m : m + 1], scale=s,
                        )
                        t1 = tmp.tile([128, BR], F32, tag="t1")
                        nc.gpsimd.tensor_scalar_mul(t1[:], a[:], 0.01)
                        nc.vector.tensor_max(tg[:], a[:], t1[:])
                    return ty, tg

                # ques embeddings
                for m in range(MC):
                    ty, tg = gated(qt, "q", bsb["bqy"], bsb["bqg"], m)
                    if m + 2 < MC:
                        wload("q", wq_d, m + 2)
                    elif m + 2 < MC + 2:
                        wload("h", wh_d, m + 2 - MC)
                    nc.vector.scalar_tensor_tensor(
                        qew[:, m, :], ty[:], watt[:, m : m + 1], tg[:],
                        op0=ALU.mult, op1=ALU.mult,
                    )
                    qe = tmp.tile([128, BR], EDT, tag="qe")
                    nc.gpsimd.tensor_mul(qe[:], ty[:], tg[:])
                    nc.gpsimd.tensor_mul(qe2[:, m, :], qe[:], qe[:])
                    if m == 0:
                        # hist-transposed activations: stream during ques phase
                        ht = big.tile([128, KC, BR], xdt, tag="ht")
                        nc.sync.dma_start(ht[:], ht_d[:])
                    if m == 2:
                        # feat inputs: stream well before the tail needs them
                        hn = big.tile([128, 2, IN], BF16, tag="hn")
                        nc.sync.dma_start(hn[:], hn_d[:])

                # hist embeddings + transposed num/den accumulation.
                # numT/denT matmuls for chunk m are emitted during chunk m+1's
                # embedding matmuls so the tensor engine never waits on the
                # vector engine's he/he2 production.
                def numden(m):
                    for g in range(2):
                        sl = slice(128 * g, 128 * (g + 1))
                        nc.tensor.matmul(
                            numT_ps[g][:], he[:, m, sl], qew[:, m, sl],
                            start=(m == 0), stop=(m == MC - 1),
                        )
                        nc.tensor.matmul(
                            denT_ps[g][:], he2[:, m, sl], qe2[:, m, sl],
                            start=(m == 0), stop=(m == MC - 1),
                        )

                for m in range(MC):
                    ty, tg = gated(ht, "h", bsb["bhy"], bsb["bhg"], m)
                    if m + 2 < MC:
                        wload("h", wh_d, m + 2)
                    nc.vector.tensor_mul(he[:, m, :], ty[:], tg[:])
                    nc.gpsimd.tensor_mul(he2[:, m, :], he[:, m, :], he[:, m, :])
                    if m > 0:
                        numden(m - 1)
                numden(MC - 1)

                # masked scores while num/den PSUM is still allocated
                sc = []
                for g in range(2):
                    sd = tmp.tile([128, 128], F32, tag="sd")
                    nc.scalar.activation(sd[:], denT_ps[g][:], ACT.Sqrt)
                    rdT = tmp.tile([128, 128], F32, tag="rdT")
                    nc.vector.reciprocal(rdT[:], sd[:])
                    sT = sm.tile([128, 128], F32, name=f"sT{g}", tag=f"sT{g}")
                    nc.vector.tensor_mul(sT[:], numT_ps[g][:], rdT[:])
                    nc.vector.tensor_add(sT[:], sT[:], maskT[:])
                    sc.append(sT)

            # attention tail: attU^T = exp(sT), row sums via a ones-column
            # matmul, 1/rowsum folded into the output copy.
            with (
                tc.tile_pool(name="psa", bufs=2, space="PSUM") as psa,
                tc.tile_pool(name="psf", bufs=2, space="PSUM") as psf,
            ):
                for g in range(2):
                    attT = sm.tile([128, 128], EDT, name=f"attT{g}", tag=f"attT{g}")
                    nc.scalar.activation(attT[:], sc[g][:], ACT.Exp)
                    rs_ps = psa.tile([128, 1], F32, tag="rs")
                    nc.tensor.matmul(rs_ps[:], attT[:], ones[:])
                    rrs = sm.tile([128, 1], F32, name=f"rrs{g}", tag=f"rrs{g}")
                    nc.vector.reciprocal(rrs[:], rs_ps[:])
                    for c2 in range(2):
                        fsb = tmp.tile([128, 1024], BF16, tag="fsb")
                        for half in range(2):
                            c = 2 * c2 + half
                            cs = slice(512 * c, 512 * (c + 1))
                            fps = psf.tile([128, 512], F32, tag="fps")
                            nc.tensor.matmul(
                                fps[:], attT[:], hn[:, g, cs],
                                start=True, stop=True,
                            )
                            dst = fsb[:, 512 * half : 512 * (half + 1)]
                            if half == 0:
                                nc.scalar.activation(
                                    dst, fps[:], ACT.Copy, scale=rrs[:]
                                )
                            else:
                                nc.vector.tensor_scalar_mul(dst, fps[:], rrs[:])
                        nc.sync.dma_start(
                            feat_d[g, :, 1024 * c2 : 1024 * (c2 + 1)], fsb[:]
                        )

    _split_multi_waits(nc)
    return nc


# ---------------------------------------------------------------------------
# Host side
# ---------------------------------------------------------------------------

_PROG_CACHE = {}


def _get_prog(mode, zero_bias):
    key = (mode, zero_bias)
    if key not in _PROG_CACHE:
        _PROG_CACHE[key] = build_program(mode, zero_bias)
    return _PROG_CACHE[key]


def _prep_shared(W_hy, b_hy, W_hg, b_hg, W_qy, b_qy, W_qg, b_qg, W_att, mode):
    fp8 = mode == "fp8"
    xnp = ml_dtypes.float8_e4m3 if fp8 else ml_dtypes.bfloat16
    ws = WSCALE if fp8 else 1.0

    def reblock(W):
        # [IN, H] -> [MC, 128, KC, 128]; Wr[m, p, k, h] = W[128k+p, 128m+h]
        return np.ascontiguousarray(
            (W * ws).reshape(KC, 128, MC, 128).transpose(2, 1, 0, 3)
        ).astype(xnp)

    def bvec(b):
        return np.ascontiguousarray(b.reshape(MC, 128).T).astype(np.float32)

    # transposed causal mask: maskT[h, q] = 0 if h <= q (same example), -inf-ish
    # otherwise; off-diagonal 32x32 blocks fully masked.
    maskT = np.full((128, 128), NEG, np.float32)
    m32T = np.where(
        np.arange(32)[:, None] <= np.arange(32)[None, :], 0.0, NEG
    ).astype(np.float32)
    for e in range(4):
        maskT[32 * e : 32 * (e + 1), 32 * e : 32 * (e + 1)] = m32T

    # [MC, 128, 2, KC, 128]
    wh = np.ascontiguousarray(np.stack([reblock(W_hy), reblock(W_hg)], axis=2))
    wq = np.ascontiguousarray(np.stack([reblock(W_qy), reblock(W_qg)], axis=2))
    consts = np.concatenate(
        [bvec(b_qy), bvec(b_qg), bvec(b_hy), bvec(b_hg), bvec(W_att), maskT],
        axis=1,
    )
    shared = {
        "wh": wh,
        "wq": wq,
        "consts": np.ascontiguousarray(consts),
    }
    return shared, xnp


def kernel(
    hist, ques, W_hy, b_hy, W_hg, b_hg, W_qy, b_qy, W_qg, b_qg, W_att, b_att,
    mode="fp8", trace=False,
):
    from concourse.bass_utils import run_bass_kernel_spmd

    hist = np.asarray(hist, np.float32)
    ques = np.asarray(ques, np.float32)
    zero_bias = all(
        not np.any(np.asarray(b)) for b in (b_hy, b_hg, b_qy, b_qg)
    )
    nc = _get_prog(mode, zero_bias)
    shared, xnp = _prep_shared(
        np.asarray(W_hy, np.float32), np.asarray(b_hy, np.float32),
        np.asarray(W_hg, np.float32), np.asarray(b_hg, np.float32),
        np.asarray(W_qy, np.float32), np.asarray(b_qy, np.float32),
        np.asarray(W_qg, np.float32), np.asarray(b_qg, np.float32),
        np.asarray(W_att, np.float32), mode,
    )

    def pmaj(x2d):
        # [BR, IN] -> [128, KC, BR]: out[p, k, b] = x2d[b, 128k+p]
        return np.ascontiguousarray(
            x2d.T.reshape(KC, 128, BR).transpose(1, 0, 2)
        ).astype(xnp)

    in_maps = []
    for c in range(NCORES):
        hs = hist[c * BL : (c + 1) * BL].reshape(BR, IN)
        qs = ques[c * BL : (c + 1) * BL].reshape(BR, IN)
        im = dict(shared)
        im["qt"] = pmaj(qs)
        im["ht"] = pmaj(hs)
        im["hn"] = np.ascontiguousarray(
            hs.reshape(2, 128, IN).transpose(1, 0, 2)
        ).astype(ml_dtypes.bfloat16)
        in_maps.append(im)

    res = run_bass_kernel_spmd(
        nc, in_maps, core_ids=list(range(NCORES)), trace=trace
    )
    feat = np.concatenate(
        [
            r["feat"].astype(np.float32).reshape(BR, IN).reshape(BL, R, IN)
            for r in res.results
        ],
        axis=0,
    )
    if trace:
        return feat, res
    return feat


# revision 17
# speedup vs baseline: 1.5715x; 1.0045x over previous
"""Trainium2 Bass kernel for nn_H_ATT (GatedTrans pair-attention block).

Math (per example):
  HE = tanh(hist@W_hy+b_hy) * lrelu(hist@W_hg+b_hg)      [R, H]
  QE = tanh(ques@W_qy+b_qy) * lrelu(ques@W_qg+b_qg)      [R, H]
  numT[h,q] = sum_k HE[h,k]*QE[q,k]*W_att[k]
  denT[h,q] = sqrt(sum_k HE[h,k]^2 * QE[q,k]^2)
  sT = numT / den                   (b_att cancels in softmax)
  attU^T = exp(sT + maskT)          (causal mask additive; unnormalized)
  feat = (attU @ hist) / rowsum(attU)    [R, 2H]

Sharding: pure data parallel, 8 examples per core on 8 NeuronCores.
Default mode quantizes the four big [IN,H] weight matrices and the
activations to fp8 e4m3 (weights pre-scaled by 1024 to clear the e4m3
subnormal range; the 1/1024 descale is folded into the tanh/relu
activations) and runs the embedding GEMMs in DoubleRow perf mode.
The score/attention path stays in bf16 with fp32 PSUM accumulation.
All host-side layouts are partition-major so every DMA is a straight
[128, contiguous-bytes] copy.
"""

import numpy as np
import ml_dtypes

import bass_rust
import concourse.bass as bass
import concourse.mybir as mybir
import concourse.tile as tile
from concourse.vector_clock import ScopedClock

# ---------------------------------------------------------------------------
# Workaround: this walrus build accepts only ONE semaphore wait on an SP
# Drain, but TileContext's tail drain carries one wait per live semaphore.
# Split them across a chain of drains.
# ---------------------------------------------------------------------------


def _patched_drain_and_barrier(self, tick_clock, wait_clock):
    nc = self.nc
    drain_inst = nc.sync.drain()
    wait_clock.add_sem_waits(
        drain_inst.ins, ScopedClock({None: tick_clock.global_clock})
    )
    waits = list(drain_inst.ins.sync_info.on_wait)
    if len(waits) > 1:
        drain_inst.ins.sync_info = bass_rust.SyncInfo(
            on_wait=waits[:1], on_update=list(drain_inst.ins.sync_info.on_update)
        )
        for i in range(1, len(waits)):
            extra = nc.sync.drain()
            extra.ins.sync_info = bass_rust.SyncInfo(
                on_wait=waits[i : i + 1], on_update=[]
            )
    nc.all_engine_barrier()
    assert self.sems is not None
    popped = nc._tile_sem_poison_stack.pop()
    assert popped is self._sem_poison
    nc.clear_and_free_semaphores(list(self.sems.allocated().values()))
    nc.all_engine_barrier()


tile.TileContext._drain_and_barrier = _patched_drain_and_barrier


def _split_multi_waits(nc):
    """This walrus build accepts at most one semaphore wait per instruction.
    Hoist extra waits onto standalone EventSemaphore instructions inserted
    just before the owning instruction in the same engine's stream."""
    uid = [0]
    for f in nc.m.functions:
        for bb in f.blocks:
            out = []
            for inst in bb.instructions:
                si = inst.sync_info
                if si is not None and len(si.on_wait) > 1:
                    waits = list(si.on_wait)
                    for w in waits[:-1]:
                        nop = mybir.InstEventSemaphore(
                            name=f"I-waitsplit-{uid[0]}", ins=[], outs=[]
                        )
                        uid[0] += 1
                        nop.engine = inst.engine
                        nop.sync_info = bass_rust.SyncInfo(
                            on_wait=[w], on_update=[]
                        )
                        out.append(nop)
                    inst.sync_info = bass_rust.SyncInfo(
                        on_wait=[waits[-1]], on_update=list(si.on_update)
                    )
                out.append(inst)
            bb.instructions[:] = out

# ---------------------------------------------------------------------------

B, R, H, IN = 64, 32, 1024, 2048
NCORES = 8
BL = B // NCORES  # examples per core
BR = BL * R  # 256 rows per core
KC = IN // 128  # 16 contraction chunks
MC = H // 128  # 8 h chunks
NEG = -1.0e30
WSCALE = 1024.0  # fp8 weight pre-scale

F32 = mybir.dt.float32
BF16 = mybir.dt.bfloat16
FP8 = mybir.dt.float8e4


def build_program(mode="fp8", zero_bias=True):
    fp8 = mode == "fp8"
    xdt = FP8 if fp8 else BF16
    s = (1.0 / WSCALE) if fp8 else 1.0
    EDT = BF16

    nc = bass.Bass()
    qt_d = nc.dram_tensor("qt", [128, KC, BR], xdt, kind="ExternalInput")
    ht_d = nc.dram_tensor("ht", [128, KC, BR], xdt, kind="ExternalInput")
    hn_d = nc.dram_tensor("hn", [128, 2, IN], BF16, kind="ExternalInput")
    wh_d = nc.dram_tensor("wh", [MC, 128, 2, KC, 128], xdt, kind="ExternalInput")
    wq_d = nc.dram_tensor("wq", [MC, 128, 2, KC, 128], xdt, kind="ExternalInput")
    # packed consts: bqy|bqg|bhy|bhg|watt (5*MC cols) then maskT (128 cols)
    consts_d = nc.dram_tensor("consts", [128, 5 * MC + 128], F32, kind="ExternalInput")
    feat_d = nc.dram_tensor("feat", [2, 128, IN], BF16, kind="ExternalOutput")

    ACT = mybir.ActivationFunctionType
    ALU = mybir.AluOpType

    with tile.TileContext(nc) as tc:
        with (
            tc.tile_pool(name="big", bufs=1) as big,
            tc.tile_pool(name="wts", bufs=4) as wts,
            tc.tile_pool(name="tmp", bufs=3) as tmp,
            tc.tile_pool(name="sm", bufs=1) as sm,
        ):
            # First-phase DMA triggers in critical-path order (SP issues
            # them serially at ~600ns each): the very first matmul chain
            # needs only the m=0 y-unit weights + the first half of qt, so
            # those land first; activations are split into half tiles so
            # the chain starts after 256KB instead of 1MB.
            pending = {}

            def wload(which, w_dram, m):
                wt = wts.tile([128, 2, KC, 128], xdt, tag="wt")
                nc.sync.dma_start(wt[:], w_dram[m])
                pending[(which, m)] = (wt[:, 0], wt[:, 1])

            wty0 = wts.tile([128, KC, 128], xdt, tag="wty0")
            nc.sync.dma_start(wty0[:], wq_d[0, :, 0])
            qtA = big.tile([128, KC // 2, BR], xdt, tag="qtA")
            nc.sync.dma_start(qtA[:], qt_d[:, : KC // 2, :])
            wtg0 = wts.tile([128, KC, 128], xdt, tag="wtg0")
            nc.sync.dma_start(wtg0[:], wq_d[0, :, 1])
            qtB = big.tile([128, KC // 2, BR], xdt, tag="qtB")
            nc.sync.dma_start(qtB[:], qt_d[:, KC // 2 :, :])
            pending[("q", 0)] = (wty0, wtg0)
            wload("q", wq_d, 1)

            consts = sm.tile([128, 5 * MC + 128], F32, tag="consts")
            nc.sync.dma_start(consts[:], consts_d[:])
            bsb = {
                n: consts[:, i * MC : (i + 1) * MC]
                for i, n in enumerate(("bqy", "bqg", "bhy", "bhg"))
            }
            watt = consts[:, 4 * MC : 5 * MC]
            maskT = consts[:, 5 * MC :]

            warm = sm.tile([128, 1], F32, tag="warm")
            nc.vector.memset(warm[:], 0.5)
            wout = sm.tile([128, 1], F32, tag="wout")
            ones = sm.tile([128, 1], BF16, tag="ones")
            nc.vector.memset(ones[:], 1.0)

            he = big.tile([128, MC, BR], EDT, tag="he")
            he2 = big.tile([128, MC, BR], EDT, tag="he2")
            qew = big.tile([128, MC, BR], EDT, tag="qew")
            qe2 = big.tile([128, MC, BR], EDT, tag="qe2")

            with (
                tc.tile_pool(name="pse", bufs=2, space="PSUM") as pse,
                tc.tile_pool(name="psnd", bufs=1, space="PSUM") as psnd,
            ):
                numT_ps = [
                    psnd.tile([128, 128], F32, name=f"num{g}", tag=f"num{g}")
                    for g in range(2)
                ]
                denT_ps = [
                    psnd.tile([128, 128], F32, name=f"den{g}", tag=f"den{g}")
                    for g in range(2)
                ]

                def embed_mm(ps, wt, xtA, xtB):
                    HK = KC // 2
                    if fp8:
                        for j in range(KC // 2):
                            xt = xtA if 2 * j < HK else xtB
                            o = 0 if 2 * j < HK else HK
                            nc.tensor.matmul(
                                ps[:],
                                wt[:, 2 * j : 2 * j + 2, :],
                                xt[:, 2 * j - o : 2 * j + 2 - o, :],
                                start=(j == 0),
                                stop=(j == KC // 2 - 1),
                                perf_mode=mybir.MatmulPerfMode.DoubleRow,
                            )
                    else:
                        for k in range(KC):
                            xt = xtA if k < HK else xtB
                            nc.tensor.matmul(
                                ps[:], wt[:, k, :], xt[:, k % HK, :],
                                start=(k == 0), stop=(k == KC - 1),
                            )

                def gated(xtA, xtB, which, by, bg, m):
                    """Consumes the prefetched weight tile for (which, m)."""
                    wty, wtg = pending.pop((which, m))
                    psy = pse.tile([128, BR], F32, tag="psy")
                    embed_mm(psy, wty, xtA, xtB)
                    psg = pse.tile([128, BR], F32, tag="psg")
                    embed_mm(psg, wtg, xtA, xtB)
                    ty = tmp.tile([128, BR], EDT, tag="ty")
                    nc.scalar.activation(
                        ty[:], psy[:], ACT.Tanh,
                        bias=(0.0 if zero_bias else by[:, m : m + 1]), scale=s,
                    )
                    tg = tmp.tile([128, BR], EDT, tag="tg")
                    if zero_bias:
                        # lrelu(s*z) = 0.01*s*z + 0.99*relu(s*z)
                        r = tmp.tile([128, BR], F32, tag="r")
                        nc.scalar.activation(r[:], psg[:], ACT.Relu, scale=0.99 * s)
                        nc.vector.scalar_tensor_tensor(
                            tg[:], psg[:], 0.01 * s, r[:],
                            op0=ALU.mult, op1=ALU.add,
                        )
                    else:
                        # lrelu(s*z + b) = max(a, 0.01a), a = s*z + b
                        a = tmp.tile([128, BR], F32, tag="r")
                        nc.scalar.activation(
                            a[:], psg[:], ACT.Identity,
                            bias=bg[:, m : m + 1], scale=s,
                        )
                        t1 = tmp.tile([128, BR], F32, tag="t1")
                        nc.gpsimd.tensor_scalar_mul(t1[:], a[:], 0.01)
                        nc.vector.tensor_max(tg[:], a[:], t1[:])
                    return ty, tg

                # ques embeddings
                for m in range(MC):
                    ty, tg = gated(qtA, qtB, "q", bsb["bqy"], bsb["bqg"], m)
                    if m + 2 < MC:
                        wload("q", wq_d, m + 2)
                    elif m + 2 < MC + 2:
                        wload("h", wh_d, m + 2 - MC)
                    nc.vector.scalar_tensor_tensor(
                        qew[:, m, :], ty[:], watt[:, m : m + 1], tg[:],
                        op0=ALU.mult, op1=ALU.mult,
                    )
                    qe = tmp.tile([128, BR], EDT, tag="qe")
                    nc.gpsimd.tensor_mul(qe[:], ty[:], tg[:])
                    nc.gpsimd.tensor_mul(qe2[:, m, :], qe[:], qe[:])
                    if m == 0:
                        # hist-transposed activations: stream during ques phase
                        htA = big.tile([128, KC // 2, BR], xdt, tag="htA")
                        nc.sync.dma_start(htA[:], ht_d[:, : KC // 2, :])
                        htB = big.tile([128, KC // 2, BR], xdt, tag="htB")
                        nc.sync.dma_start(htB[:], ht_d[:, KC // 2 :, :])

                # hist embeddings + transposed num/den accumulation.
                # numT/denT matmuls for chunk m are emitted during chunk m+1's
                # embedding matmuls so the tensor engine never waits on the
                # vector engine's he/he2 production.
                def numden(m):
                    for g in range(2):
                        sl = slice(128 * g, 128 * (g + 1))
                        nc.tensor.matmul(
                            numT_ps[g][:], he[:, m, sl], qew[:, m, sl],
                            start=(m == 0), stop=(m == MC - 1),
                        )
                        nc.tensor.matmul(
                            denT_ps[g][:], he2[:, m, sl], qe2[:, m, sl],
                            start=(m == 0), stop=(m == MC - 1),
                        )

                for m in range(MC):
                    ty, tg = gated(htA, htB, "h", bsb["bhy"], bsb["bhg"], m)
                    if m + 2 < MC:
                        wload("h", wh_d, m + 2)
                    nc.vector.tensor_mul(he[:, m, :], ty[:], tg[:])
                    nc.gpsimd.tensor_mul(he2[:, m, :], he[:, m, :], he[:, m, :])
                    if m == 0:
                        # feat inputs: hist phase has DMA slack, ques doesn't
                        hn = big.tile([128, 2, IN], BF16, tag="hn")
                        nc.sync.dma_start(hn[:], hn_d[:])
                    if m == 6:
                        # preload the Sqrt activation table off the critical
                        # path (ACT_TABLE_LOAD costs ~1.3us; the engine seems
                        # to keep the Tanh/Relu table plus one swap slot, so
                        # Exp is preloaded separately after the sqrt uses)
                        nc.scalar.activation(wout[:], warm[:], ACT.Sqrt)
                    if m > 0:
                        numden(m - 1)
                numden(MC - 1)

                # masked scores while num/den PSUM is still allocated
                sc = []
                for g in range(2):
                    sd = tmp.tile([128, 128], F32, tag="sd")
                    nc.scalar.activation(sd[:], denT_ps[g][:], ACT.Sqrt)
                    rdT = tmp.tile([128, 128], F32, tag="rdT")
                    nc.vector.reciprocal(rdT[:], sd[:])
                    sT = sm.tile([128, 128], F32, name=f"sT{g}", tag=f"sT{g}")
                    nc.vector.tensor_mul(sT[:], numT_ps[g][:], rdT[:])
                    nc.vector.tensor_add(sT[:], sT[:], maskT[:])
                    sc.append(sT)
                # preload the Exp table while vector finishes the scores
                nc.scalar.activation(wout[:], warm[:], ACT.Exp)

            # attention tail: attU^T = exp(sT), row sums via a ones-column
            # matmul, 1/rowsum folded into the output copy.
            with (
                tc.tile_pool(name="psa", bufs=2, space="PSUM") as psa,
                tc.tile_pool(name="psf", bufs=2, space="PSUM") as psf,
            ):
                for g in range(2):
                    attT = sm.tile([128, 128], EDT, name=f"attT{g}", tag=f"attT{g}")
                    nc.scalar.activation(attT[:], sc[g][:], ACT.Exp)
                    rs_ps = psa.tile([128, 1], F32, tag="rs")
                    nc.tensor.matmul(rs_ps[:], attT[:], ones[:])
                    rrs = sm.tile([128, 1], F32, name=f"rrs{g}", tag=f"rrs{g}")
                    nc.vector.reciprocal(rrs[:], rs_ps[:])
                    for c2 in range(2):
                        fsb = tmp.tile([128, 1024], BF16, tag="fsb")
                        for half in range(2):
                            c = 2 * c2 + half
                            cs = slice(512 * c, 512 * (c + 1))
                            fps = psf.tile([128, 512], F32, tag="fps")
                            nc.tensor.matmul(
                                fps[:], attT[:], hn[:, g, cs],
                                start=True, stop=True,
                            )
                            dst = fsb[:, 512 * half : 512 * (half + 1)]
                            if half == 0:
                                nc.scalar.activation(
                                    dst, fps[:], ACT.Copy, scale=rrs[:]
                                )
                            else:
                                nc.vector.tensor_scalar_mul(dst, fps[:], rrs[:])
                        nc.sync.dma_start(
                            feat_d[g, :, 1024 * c2 : 1024 * (c2 + 1)], fsb[:]
                        )

    _split_multi_waits(nc)
    return nc


# ---------------------------------------------------------------------------
# Host side
# ---------------------------------------------------------------------------

_PROG_CACHE = {}


def _get_prog(mode, zero_bias):
    key = (mode, zero_bias)
    if key not in _PROG_CACHE:
        _PROG_CACHE[key] = build_program(mode, zero_bias)
    return _PROG_CACHE[key]


def _prep_shared(W_hy, b_hy, W_hg, b_hg, W_qy, b_qy, W_qg, b_qg, W_att, mode):
    fp8 = mode == "fp8"
    xnp = ml_dtypes.float8_e4m3 if fp8 else ml_dtypes.bfloat16
    ws = WSCALE if fp8 else 1.0

    def reblock(W):
        # [IN, H] -> [MC, 128, KC, 128]; Wr[m, p, k, h] = W[128k+p, 128m+h]
        return np.ascontiguousarray(
            (W * ws).reshape(KC, 128, MC, 128).transpose(2, 1, 0, 3)
        ).astype(xnp)

    def bvec(b):
        return np.ascontiguousarray(b.reshape(MC, 128).T).astype(np.float32)

    # transposed causal mask: maskT[h, q] = 0 if h <= q (same example), -inf-ish
    # otherwise; off-diagonal 32x32 blocks fully masked.
    maskT = np.full((128, 128), NEG, np.float32)
    m32T = np.where(
        np.arange(32)[:, None] <= np.arange(32)[None, :], 0.0, NEG
    ).astype(np.float32)
    for e in range(4):
        maskT[32 * e : 32 * (e + 1), 32 * e : 32 * (e + 1)] = m32T

    # [MC, 128, 2, KC, 128]
    wh = np.ascontiguousarray(np.stack([reblock(W_hy), reblock(W_hg)], axis=2))
    wq = np.ascontiguousarray(np.stack([reblock(W_qy), reblock(W_qg)], axis=2))
    consts = np.concatenate(
        [bvec(b_qy), bvec(b_qg), bvec(b_hy), bvec(b_hg), bvec(W_att), maskT],
        axis=1,
    )
    shared = {
        "wh": wh,
        "wq": wq,
        "consts": np.ascontiguousarray(consts),
    }
    return shared, xnp


def kernel(
    hist, ques, W_hy, b_hy, W_hg, b_hg, W_qy, b_qy, W_qg, b_qg, W_att, b_att,
    mode="fp8", trace=False,
):
    from concourse.bass_utils import run_bass_kernel_spmd

    hist = np.asarray(hist, np.float32)
    ques = np.asarray(ques, np.float32)
    zero_bias = all(
        not np.any(np.asarray(b)) for b in (b_hy, b_hg, b_qy, b_qg)
    )
    nc = _get_prog(mode, zero_bias)
    shared, xnp = _prep_shared(
        np.asarray(W_hy, np.float32), np.asarray(b_hy, np.float32),
        np.asarray(W_hg, np.float32), np.asarray(b_hg, np.float32),
        np.asarray(W_qy, np.float32), np.asarray(b_qy, np.float32),
        np.asarray(W_qg, np.float32), np.asarray(b_qg, np.float32),
        np.asarray(W_att, np.float32), mode,
    )

    def pmaj(x2d):
        # [BR, IN] -> [128, KC, BR]: out[p, k, b] = x2d[b, 128k+p]
        return np.ascontiguousarray(
            x2d.T.reshape(KC, 128, BR).transpose(1, 0, 2)
        ).astype(xnp)

    in_maps = []
    for c in range(NCORES):
        hs = hist[c * BL : (c + 1) * BL].reshape(BR, IN)
        qs = ques[c * BL : (c + 1) * BL].reshape(BR, IN)
        im = dict(shared)
        im["qt"] = pmaj(qs)
        im["ht"] = pmaj(hs)
        im["hn"] = np.ascontiguousarray(
            hs.reshape(2, 128, IN).transpose(1, 0, 2)
        ).astype(ml_dtypes.bfloat16)
        in_maps.append(im)

    res = run_bass_kernel_spmd(
        nc, in_maps, core_ids=list(range(NCORES)), trace=trace
    )
    feat = np.concatenate(
        [
            r["feat"].astype(np.float32).reshape(BR, IN).reshape(BL, R, IN)
            for r in res.results
        ],
        axis=0,
    )
    if trace:
        return feat, res
    return feat
